# revision 39
# baseline (speedup 1.0000x reference)
"""BlockNet Trainium2 kernel: data-parallel over 8 NeuronCores.

v3 layout (per core, batch Nb=256):
- Host pre-transposes x to bf16 xprep[(w,c), h, b]; slab row 0 is a
  constant-1.0 row so every bias folds into the matmuls.
- Fused matmul per (j-group, i, kh): lhsT [K, 96] block-diag over j,
  columns [untied-minus-shared | shared | gate] each padded to 32
  (engine APs only address partition windows at 0/32/64/96), rhs =
  slab [K, 256] at h=s*i+kh, accumulated over kh into PSUM
  acc[96, G*256] (G=4 output rows -> wide blend ops).
- Blend y = relu(s + sigmoid(g)*d): ACT sigmoid; DVE mult;
  add on KADD in {dve, cce (DMA-accumulate into PSUM), pe (identity
  matmul accumulate)}; relu on KRELU in {dve (tensor_scalar 4x), act}.
- All compute bf16 (host casts), PSUM f32.
"""
import os
import numpy as np
import ml_dtypes

KSTAGE = int(os.environ.get('KSTAGE', '4'))
KADD = os.environ.get('KADD', 'dve')      # dve | cce | pe
KRELU = os.environ.get('KRELU', 'dve')    # dve | act | mix
KG = int(os.environ.get('KG', '4'))       # i-group size

import concourse.bass as bass
import concourse.mybir as mybir
import concourse.bacc as bacc
import concourse.tile as tile
from concourse.bass_utils import run_bass_kernel_spmd

N_CORES = 8
NB = 256
BATCH = 2048
BF16 = mybir.dt.bfloat16
F32 = mybir.dt.float32

# (cin, cout, k, s, oh, iw) per block
CFG = [(3, 4, 5, 3, 20, 64), (4, 6, 3, 2, 9, 20), (6, 16, 3, 2, 4, 9),
       (16, 32, 3, 2, 1, 4)]
JGS = [[(0, 8), (8, 8), (16, 4)], [(0, 5), (5, 4)], [(0, 2), (2, 2)],
       [(0, 1)]]
# K per (block, jg) INCLUDING the ones row 0.
KWIN = [[79, 79, 43], [76, 108], [62, 88], [80]]
# slab start row in xprep for b1 j-groups (w0*cin)
SROW1 = [0, 72, 144]


def _igroups(oh, g):
    return [(i0, min(g, oh - i0)) for i0 in range(0, oh, g)]


def _row_of(blk, w, c):
    """Data row of (w, c) in block blk's input Y (before +1 ones shift)."""
    cin = CFG[blk][0]
    if blk == 2:
        return w * 6 + c if w < 5 else 32 + (w - 5) * 6 + c
    return w * cin + c


_CACHE = {}


def _build():
    nc = bacc.Bacc("TRN2", target_bir_lowering=False, debug=False,
                   num_devices=N_CORES)
    xprep = nc.dram_tensor("xprep", [192, 64, NB], BF16,
                           kind="ExternalInput").ap()
    # aux row 0 = ones, rows 1-31 = zeros
    onesrow = nc.dram_tensor("onesrow", [32, 64, NB], BF16,
                             kind="ExternalInput").ap()
    wb = {}
    for blk in range(4):
        cin, cout, k, s, oh, iw = CFG[blk]
        for jg in range(len(JGS[blk])):
            K = KWIN[blk][jg]
            wb[(blk, jg)] = nc.dram_tensor(
                f"wb{blk}_{jg}", [K, oh, k, 96], BF16,
                kind="ExternalInput").ap()
    wfc = nc.dram_tensor("wfc", [64, 4], BF16, kind="ExternalInput").ap()
    out_d = nc.dram_tensor("out", [4, NB], F32, kind="ExternalOutput").ap()
    eye = None
    if KADD in ('pe', 'mix'):
        eye = nc.dram_tensor("eye32", [32, 32], BF16,
                             kind="ExternalInput").ap()

    with tile.TileContext(nc) as tc:
        import contextlib
        ctx = contextlib.ExitStack()
        with ctx:
            pconst = ctx.enter_context(tc.tile_pool(name="const", bufs=1))
            pslab = ctx.enter_context(tc.tile_pool(name="slab", bufs=1))
            pw1 = ctx.enter_context(tc.tile_pool(name="w1", bufs=6))
            pwS = ctx.enter_context(tc.tile_pool(name="wS", bufs=1))
            pg = ctx.enter_context(tc.tile_pool(name="g", bufs=3))
            pq = ctx.enter_context(tc.tile_pool(name="q", bufs=3))
            py = ctx.enter_context(tc.tile_pool(name="y", bufs=2))
            pps = ctx.enter_context(tc.tile_pool(name="ps", bufs=4,
                                                 space="PSUM"))

            wfc_t = pconst.tile([64, 4], BF16, tag="wfc")
            nc.sync.dma_start(wfc_t[:], wfc[:])
            eye_t = None
            if KADD in ('pe', 'mix'):
                eye_t = pconst.tile([32, 32], BF16, tag="eye")
                nc.sync.dma_start(eye_t[:], eye[:])

            # block1 slabs [K, 64, NB]: row 0 = ones, rows 1.. = x window.
            # DMA issue order is tuned for startup: Y-inits (small) first,
            # then slab jg2 (smallest) + its first weight tiles, then the
            # remaining slabs, so the first matmul group's inputs land ~6us
            # in instead of queueing behind every slab chunk.
            slabs1 = []
            wS = {}
            wts = {}
            for jg in range(3):
                K = KWIN[0][jg]
                chunks = []
                for ct in range(4):
                    sc = pslab.tile([K, 16, NB], BF16,
                                    tag=f"slab{jg}_{ct}",
                                    name=f"slab{jg}_{ct}")
                    chunks.append(sc)
                slabs1.append(chunks)

            def _wt_dma(jg, i0, G):
                if (jg, i0) in wts:
                    return
                wt = pw1.tile([79, KG, 5, 96], BF16, tag="w1", name="wt")
                nc.gpsimd.dma_start(wt[0:KWIN[0][jg], 0:G, :, :],
                                    wb[(0, jg)][:, i0:i0 + G, :, :])
                wts[(jg, i0)] = wt


            # pool(SWDGE) queue carries slab chunks + b1 weights in exact
            # consumption order; sync(HWDGE) carries Y-inits + small weights
            def _slab_wt(jg, nsync=0):
                K = KWIN[0][jg]
                _wt_dma(jg, 0, 4)  # head start over the 2.2us chunk gen
                for ct in range(4):
                    sc = slabs1[jg][ct]
                    nc.gpsimd.dma_start(sc[0:1, :, :],
                                        onesrow[0:1, 0:16, :])
                    eng = nc.sync if ct < nsync else nc.gpsimd
                    eng.dma_start(
                        sc[1:K, :, :],
                        xprep[SROW1[jg]:SROW1[jg] + K - 1,
                              16 * ct:16 * (ct + 1), :])
                    _wt_dma(jg, 4 * ct, 4)
                _wt_dma(jg, 16, 4)

            # wt(0,0)'s SWDGE gen precedes the first slab chunk's 2.2us
            # gen; jg0's first two chunks ride the idle sync/HWDGE queue
            _wt_dma(0, 0, 4)
            _slab_wt(0, nsync=2)

            # Y tensors (next blocks' slabs): row 0 = ones, rows 1-31
            # zero pad (so blend writes start at partition 32: engine APs
            # only address partition windows at 0/32/64/96), data from 32.
            Y1 = [pslab.tile([112, 4, NB], BF16, tag=f"Y1_{c}",
                             name=f"Y1_{c}") for c in range(5)]
            Y2 = pslab.tile([88, 9, NB], BF16, tag="Y2")
            Y3 = pslab.tile([96, 4, NB], BF16, tag="Y3")
            y4 = pslab.tile([64, NB], BF16, tag="y4")
            for c in range(5):
                nc.sync.dma_start(Y1[c][0:32, :, :],
                                  onesrow[0:32, 0:4, :])
            nc.sync.dma_start(Y2[0:32, :, :], onesrow[0:32, 0:9, :])
            # Y2 data rows 62,63 are padding never written by blends; zero
            # them so 0-weights never multiply stale NaNs.
            nc.sync.dma_start(Y2[62:64, :, :], onesrow[30:32, 0:9, :])
            nc.sync.dma_start(Y3[0:32, :, :], onesrow[0:32, 0:4, :])
            nc.sync.dma_start(y4[0:32, :], onesrow[0:32, 0, :])
            _slab_wt(1, nsync=NS1)
            _slab_wt(2, nsync=NS1)

            # blocks 2-4 weights: bulk one-DMA-per-(blk,jg) loads
            for blk in range(1, 4):
                cin, cout, k, s, oh, iw = CFG[blk]
                for jg in range(len(JGS[blk])):
                    K = KWIN[blk][jg]
                    t = pwS.tile([K, oh, k, 96], BF16, tag=f"wS{blk}_{jg}")
                    nc.sync.dma_start(t[:], wb[(blk, jg)][:])
                    wS[(blk, jg)] = t

            nblend = [0]

            def blend(blk, jg, i0, G, acc, ytensor, rowbase, nrows):
                g_t = pg.tile([32, KG, NB], BF16, tag="g")
                q_t = pq.tile([32, KG, NB], BF16, tag="q")
                gs = g_t[:, 0:G, :]
                qs = q_t[:, 0:G, :]
                nc.scalar.activation(gs, acc[64:96, 0:G, :],
                                     mybir.ActivationFunctionType.Sigmoid)
                if KCOPYD:
                    # drain d to bf16 SBUF on ACT so the DVE mult runs in
                    # 2x mode (both operands bf16/SBUF) instead of 1x PSUM
                    d_t = pd.tile([32, KG, NB], BF16, tag="d")
                    ds = d_t[:, 0:G, :]
                    nc.scalar.activation(ds, acc[0:32, 0:G, :],
                                         mybir.ActivationFunctionType.Copy)
                    nc.vector.tensor_mul(qs, ds, gs)
                else:
                    nc.vector.tensor_mul(qs, acc[0:32, 0:G, :], gs)
                nadd = nblend[0]
                addeng = KADD if KADD != 'mix' else \
                    ('pe' if nadd % 2 else 'dve')
                if addeng == 'pe':
                    for gi in range(G):
                        nc.tensor.matmul(acc[32:64, gi, :], eye_t[:],
                                         q_t[:, gi, :], start=False,
                                         stop=True)
                    src = acc[32:64, 0:G, :]
                elif addeng == 'cce':
                    nc.gpsimd.dma_start(acc[32:64, 0:G, :], qs,
                                        accum_op=mybir.AluOpType.add)
                    src = acc[32:64, 0:G, :]
                else:
                    y_t = pd.tile([32, KG, NB], BF16, tag="yt")
                    ys = y_t[:, 0:G, :]
                    nc.vector.tensor_add(ys, qs, acc[32:64, 0:G, :])
                    src = ys
                if ytensor is y4:
                    dst = ytensor[32 + rowbase:32 + rowbase + nrows, :]
                    src = src[0:nrows, 0, :]
                elif isinstance(ytensor, list):
                    dst = ytensor[i0 // 4][32 + rowbase:32 + rowbase + nrows,
                                           0:G, :]
                    src = src[0:nrows]
                else:
                    dst = ytensor[32 + rowbase:32 + rowbase + nrows,
                                  i0:i0 + G, :]
                    src = src[0:nrows]
                n = nblend[0]
                nblend[0] += 1
                eng = KRELU if KRELU != 'mix' else ('act' if n % 2 else 'dve')
                if eng == 'act':
                    nc.scalar.activation(dst, src,
                                         mybir.ActivationFunctionType.Relu)
                else:
                    nc.vector.tensor_scalar_max(dst, src, 0.0)

            # ---- block 1 ----
            cin, cout, k, s, oh, iw = CFG[0]
            for jg in (0, 1, 2):
                j0, nj = JGS[0][jg]
                K = KWIN[0][jg]
                for (i0, G) in _igroups(oh, KG):
                    if (jg, i0) not in wts:
                        _wt_dma(jg, i0, G)
                    wt = wts[(jg, i0)]
                    acc = pps.tile([96, KG, NB], F32, tag="acc")
                    for gi in range(G):
                        i = i0 + gi
                        for kh in range(k):
                            h = s * i + kh
                            nc.tensor.matmul(
                                acc[:, gi, :], wt[0:K, gi, kh, :],
                                slabs1[jg][h // 16][0:K, h % 16, :],
                                start=(kh == 0), stop=(kh == k - 1))
                    blend(0, jg, i0, G, acc, Y1, 32 * jg, nj * cout)

            # ---- blocks 2-4 ----
            srcs = [Y1, Y2, Y3]
            outs = [Y2, Y3, y4]
            for blk in range(1, min(KSTAGE, 4)):
                cin, cout, k, s, oh, iw = CFG[blk]
                src = srcs[blk - 1]
                for jg, (j0, nj) in enumerate(JGS[blk]):
                    K = KWIN[blk][jg]
                    for (i0, G) in _igroups(oh, KG):
                        acc = pps.tile([96, KG, NB], F32, tag="acc")
                        for gi in range(G):
                            i = i0 + gi
                            for kh in range(k):
                                h = s * i + kh
                                if isinstance(src, list):
                                    rhs = src[h // 4][0:K, h % 4, :]
                                else:
                                    rhs = src[0:K, h, :]
                                nc.tensor.matmul(
                                    acc[:, gi, :],
                                    wS[(blk, jg)][:, i, kh, :],
                                    rhs, start=(kh == 0),
                                    stop=(kh == k - 1))
                        blend(blk, jg, i0, G, acc, outs[blk - 1],
                              32 * jg, nj * cout)

            # ---- FC (bias folded via y4 ones row) ----
            accfct = pps.tile([96, KG, NB], F32, tag="acc")
            accfc = accfct[0:4, 0, :]
            nc.tensor.matmul(accfc, wfc_t[:], y4[:], start=True, stop=True)
            out_t = pconst.tile([4, NB], F32, tag="outt")
            nc.scalar.activation(out_t[:], accfc,
                                 mybir.ActivationFunctionType.Identity)
            nc.sync.dma_start(out_d[:], out_t[:])

    nc.compile()
    return nc


def _prep_weights(inputs):
    """Pack wb{blk}_{jg} [K, oh, k, 96] bf16 with biases in row 0 (kh=0).
    Columns: [d=untied-shared | shared | gate] blocks padded to 32."""
    arrs = {}
    for blk in range(4):
        cin, cout, k, st, oh, iw = CFG[blk]
        L = oh * oh
        ln = cin * k * k
        wu = np.asarray(inputs[f"w_uc{blk + 1}"], np.float32).reshape(
            L, ln, cout)
        bu = np.asarray(inputs[f"b_uc{blk + 1}"], np.float32)[0]
        wp = np.asarray(inputs[f"w_pc{blk + 1}"], np.float32)
        bp = np.asarray(inputs[f"b_pc{blk + 1}"], np.float32)
        wg = np.asarray(inputs[f"w_wl{blk + 1}"], np.float32)[0]
        bg = np.asarray(inputs[f"b_wl{blk + 1}"], np.float32)[0]

        for jg, (j0, nj) in enumerate(JGS[blk]):
            K = KWIN[blk][jg]
            W = np.zeros((K, oh, k, 96), np.float32)
            w0_slab = st * j0 if blk == 0 else 0
            for jt in range(nj):
                j = j0 + jt
                co = jt * cout
                W[0, :, 0, co:co + cout] = bu[:, :, j].T - bp[None, :]
                W[0, :, 0, 32 + co:32 + co + cout] = bp[None, :]
                W[0, :, 0, 64 + co:64 + co + cout] = bg
                for kw in range(k):
                    w = st * j + kw
                    for c in range(cin):
                        if blk == 0:
                            row = 1 + (w - w0_slab) * cin + c
                        else:
                            row = 32 + _row_of(blk, w, c)
                        kidx = c * k * k
                        for kh in range(k):
                            un = wu[np.arange(oh) * oh + j,
                                    kidx + kh * k + kw, :]
                            W[row, :, kh, co:co + cout] = (
                                un - wp[:, c, kh, kw][None, :])
                            W[row, :, kh, 32 + co:32 + co + cout] = \
                                wp[:, c, kh, kw][None, :]
                            W[row, :, kh, 64 + co:64 + co + cout] = \
                                wg[c, kh, kw]
            arrs[f"wb{blk}_{jg}"] = W.astype(ml_dtypes.bfloat16)

    wfc = np.zeros((64, 4), np.float32)
    wfc[0] = np.asarray(inputs["fc_b"], np.float32)
    wfc[32:] = np.asarray(inputs["fc_w"], np.float32)
    arrs["wfc"] = wfc.astype(ml_dtypes.bfloat16)
    aux = np.zeros((32, 64, NB), ml_dtypes.bfloat16)
    aux[0] = 1.0
    arrs["onesrow"] = aux
    if KADD in ('pe', 'mix'):
        arrs["eye32"] = np.eye(32, dtype=ml_dtypes.bfloat16)
    return arrs


def _prep_x_core(x, ci):
    xc = x[ci * NB:(ci + 1) * NB]                       # [256,3,64,64]
    xprep = np.ascontiguousarray(
        xc.transpose(3, 1, 2, 0)).reshape(192, 64, NB)  # [(w,c),h,b]
    return {"xprep": xprep.astype(ml_dtypes.bfloat16)}


def kernel(**inputs):
    if "nc" not in _CACHE:
        _CACHE["nc"] = _build()
    nc = _CACHE["nc"]
    warrs = _prep_weights(inputs)
    x = np.asarray(inputs["x"], np.float32)
    in_maps = []
    for ci in range(N_CORES):
        m = _prep_x_core(x, ci)
        m.update(warrs)
        in_maps.append(m)
    res = run_bass_kernel_spmd(nc, in_maps, core_ids=list(range(N_CORES)))
    out = np.concatenate([res.results[c]["out"].T for c in range(N_CORES)],
                         axis=0)
    return out.astype(np.float32)
